# revision 1
# baseline (speedup 1.0000x reference)
"""Trainium2 Bass kernel for dynamic-LKA (CondConv depthwise mix) module.

Reference computation (per sample):
  r0 = sigmoid(mean_hw(x) @ r0_w.T + r0_b)            # [K] routing
  wk0 = sum_k r0_k * w0[k]                            # mixed 5x5 depthwise kernel
  a1 = gelu(dwconv5x5(x, wk0, pad=2, dil=1) + b0)
  r1 = sigmoid(mean_hw(a1) @ r1_w.T + r1_b)
  wk1 = sum_k r1_k * w1[k]                            # mixed 7x7 dil3 kernel
  a2 = gelu(dwconv7x7d3(a1, wk1, pad=9, dil=3) + b1)
  attn = a2 conv1x1 wp + bp
  out = x * attn

Sharding: pure data parallel, 1 sample per NeuronCore (B=8 over 8 cores).

Per-core strategy:
  - Layout: partitions p = wh*64 + c (w-half, channel); free dims (h, w_local).
    The host pre-pads/casts x into this layout so every DMA is contiguous.
  - Depthwise conv taps run as PE matmuls with *diagonal* stationary
    matrices diag(wk[:, tap]) accumulating in PSUM (1 moving column/cycle
    @2.4GHz); a fraction of h-tiles instead run on the DVE as fp32
    scalar_tensor_tensor MAC chains so both engines stay busy.
  - gelu (+channel bias) runs on the ACT engine straight out of PSUM and
    its accum_out provides the per-partition sums for the second routing.
  - 1x1 conv is one PE matmul per tile with a block-diagonal wp.
  - Final gate multiply reads a host-provided fp32 copy of x.
"""

import os
import sys
import threading

import numpy as np

for _p in ("/opt/trn_rl_repo",):
    if _p not in sys.path and os.path.isdir(_p):
        sys.path.insert(0, _p)

import concourse.bacc as bacc
import concourse.bass as bass
import concourse.mybir as mybir
import concourse.tile as tile
from concourse.bass_utils import run_bass_kernel_spmd

B, C, H, W = 8, 64, 256, 256
K = 3
NCORES = 8
WH = W // 2  # 128, per-partition w width
P = 128

F32 = mybir.dt.float32
F16 = mybir.dt.float16

TAPS5 = [(di, dj) for di in range(5) for dj in range(5)]   # conv1, offsets di-2, dj-2
TAPS7 = [(di, dj) for di in range(7) for dj in range(7)]   # conv2, offsets 3*(di-3), 3*(dj-3)
NT5, NT7 = len(TAPS5), len(TAPS7)

HTILE = 4                      # output h rows per tile -> N=512 moving columns
NTILES = H // HTILE            # 64

# x16 padded slab: 2 pad rows/cols each side (conv1 radius 2)
XPR, XPC = H + 4, WH + 4       # 260 x 132
# attn1 padded slab: 9 pad rows/cols each side (conv2 reach 9)
APR, APC = H + 18, WH + 18     # 274 x 146

# which tiles run on DVE instead of PE (load balancing)
DVE_A = frozenset(i for i in range(NTILES) if i % 15 in (1, 5, 9, 13))   # ~17
DVE_B = frozenset(i for i in range(NTILES) if i % 17 in (1, 5, 9, 13))   # ~15

ALU = mybir.AluOpType
ACTF = mybir.ActivationFunctionType


def _build_program(reps=1):
    nc = bacc.Bacc(None, target_bir_lowering=False)

    # ---- kernel I/O (host-prepped layouts) -------------------------------
    xh_d = nc.dram_tensor("xh", [P, XPR, XPC], F16, kind="ExternalInput")
    x32_d = nc.dram_tensor("x32", [P, H, WH], F32, kind="ExternalInput")
    wexp0_d = nc.dram_tensor("wexp0", [P, K, NT5], F32, kind="ExternalInput")
    wexp1_d = nc.dram_tensor("wexp1", [P, K, NT7], F32, kind="ExternalInput")
    r0wT_d = nc.dram_tensor("r0wT", [C, K], F32, kind="ExternalInput")
    r1wT_d = nc.dram_tensor("r1wT", [C, K], F32, kind="ExternalInput")
    r0b_d = nc.dram_tensor("r0b", [K, 1], F32, kind="ExternalInput")
    r1b_d = nc.dram_tensor("r1b", [K, 1], F32, kind="ExternalInput")
    s2_d = nc.dram_tensor("s2", [P, C], F32, kind="ExternalInput")
    i128_d = nc.dram_tensor("i128", [P, P], F16, kind="ExternalInput")
    wpbd_d = nc.dram_tensor("wpbd", [P, P], F16, kind="ExternalInput")
    b0_d = nc.dram_tensor("b0r", [P, 1], F32, kind="ExternalInput")
    b1_d = nc.dram_tensor("b1r", [P, 1], F32, kind="ExternalInput")
    bp_d = nc.dram_tensor("bpr", [P, 1], F32, kind="ExternalInput")
    out_d = nc.dram_tensor("out", [P, H, WH], F32, kind="ExternalOutput")

    # DRAM bounce buffers for broadcasting routing weights to all partitions
    r0scr = nc.dram_tensor("r0scr", [K, 1], F32)
    r1scr = nc.dram_tensor("r1scr", [K, 1], F32)

    with tile.TileContext(nc) as tc, \
            tc.tile_pool(name="consts", bufs=1) as consts, \
            tc.tile_pool(name="a1pool", bufs=1) as a1pool, \
            tc.tile_pool(name="smalls", bufs=1) as smalls, \
            tc.tile_pool(name="psumA", bufs=4, space="PSUM") as psumA, \
            tc.tile_pool(name="psumB", bufs=2, space="PSUM") as psumB, \
            tc.tile_pool(name="psumT", bufs=1, space="PSUM") as psumT:

        # ---- constants ----------------------------------------------------
        s2sb = consts.tile([P, C], F32)
        nc.sync.dma_start(out=s2sb, in_=s2_d[:, :])
        i128sb = consts.tile([P, P], F16)
        nc.sync.dma_start(out=i128sb, in_=i128_d[:, :])
        wpbdsb = consts.tile([P, P], F16)
        nc.sync.dma_start(out=wpbdsb, in_=wpbd_d[:, :])
        b0sb = consts.tile([P, 1], F32)
        nc.sync.dma_start(out=b0sb, in_=b0_d[:, :])
        b1sb = consts.tile([P, 1], F32)
        nc.sync.dma_start(out=b1sb, in_=b1_d[:, :])
        bpsb = consts.tile([P, 1], F32)
        nc.sync.dma_start(out=bpsb, in_=bp_d[:, :])
        r0wTsb = consts.tile([C, K], F32)
        nc.sync.dma_start(out=r0wTsb, in_=r0wT_d[:, :])
        r1wTsb = consts.tile([C, K], F32)
        nc.sync.dma_start(out=r1wTsb, in_=r1wT_d[:, :])
        r0bsb = consts.tile([K, 1], F32)
        nc.sync.dma_start(out=r0bsb, in_=r0b_d[:, :])
        r1bsb = consts.tile([K, 1], F32)
        nc.sync.dma_start(out=r1bsb, in_=r1b_d[:, :])
        wexp0sb = consts.tile([P, K, NT5], F32)
        nc.sync.dma_start(out=wexp0sb, in_=wexp0_d[:, :, :])
        wexp1sb = consts.tile([P, K, NT7], F32)
        nc.sync.dma_start(out=wexp1sb, in_=wexp1_d[:, :, :])

        # attn1 resident slab (fp16), with 9-wide zero pads/halos
        attn1 = a1pool.tile([P, APR, APC], F16)
        nc.vector.memset(attn1[:, 0:9, :], 0.0)
        nc.vector.memset(attn1[:, APR - 9:APR, :], 0.0)
        nc.vector.memset(attn1[0:C, 9:APR - 9, 0:9], 0.0)          # wh=0 left edge
        nc.vector.memset(attn1[C:P, 9:APR - 9, APC - 9:APC], 0.0)  # wh=1 right edge

        stats1 = smalls.tile([P, NTILES], F32)
        pool1raw = smalls.tile([P, 1], F32)
        pool2raw = smalls.tile([P, 1], F32)
        poolm = smalls.tile([C, 1], F32)
        poolm2 = smalls.tile([C, 1], F32)
        rsb0 = smalls.tile([K, 1], F32)
        rsb1 = smalls.tile([K, 1], F32)
        r0bc = smalls.tile([P, K], F32)
        r1bc = smalls.tile([P, K], F32)
        wk1 = smalls.tile([P, NT7], F32)
        diag1 = smalls.tile([P, NT7, P], F16)
        hgat = smalls.tile([P, H, 9], F16)   # halo exchange staging (gather)
        hswp = smalls.tile([P, H, 9], F16)   # halo exchange staging (swapped)

        def routing_chain(poolraw, scale, rwTsb, rbsb, rsb, rscr_d, rbc, pm):
            """poolraw [P,1] -> r [K] -> broadcast to all partitions [P,K]."""
            ps1 = psumT.tile([C, 1], F32)
            nc.tensor.matmul(ps1[:, :], lhsT=s2sb[:, :], rhs=poolraw[:, :],
                             start=True, stop=True)
            nc.scalar.activation(out=pm[:, :], in_=ps1[:, :],
                                 func=ACTF.Copy, bias=0.0, scale=scale)
            ps2 = psumT.tile([K, 1], F32)
            nc.tensor.matmul(ps2[:, :], lhsT=rwTsb[:, :], rhs=pm[:, :],
                             start=True, stop=True)
            nc.scalar.activation(out=rsb[:, :], in_=ps2[:, :],
                                 func=ACTF.Sigmoid, bias=rbsb[:, :], scale=1.0)
            nc.sync.dma_start(out=rscr_d[:, :], in_=rsb[:, :])
            bcast = bass.AP(tensor=rscr_d, offset=0, ap=[[0, P], [1, K]])
            nc.gpsimd.dma_start(out=rbc[:, :], in_=bcast)

        def mix_weights(rbc, wexpsb, wk):
            nc.vector.tensor_scalar(wk[:, :], wexpsb[:, 0, :], rbc[:, 0:1], None,
                                    ALU.mult)
            for k in range(1, K):
                nc.vector.scalar_tensor_tensor(wk[:, :], wexpsb[:, k, :],
                                               rbc[:, k:k + 1], wk[:, :],
                                               ALU.mult, ALU.add)

        def build_diags(diag, wk, ntaps):
            for t in range(ntaps):
                nc.vector.tensor_scalar(diag[:, t, :], i128sb[:, :],
                                        wk[:, t:t + 1], None, ALU.mult)

        # ============ phases (repeated `reps` times for timing runs) =======
        for _rep in range(reps):
            with tc.tile_pool(name="xpool", bufs=1) as xpool, \
                    tc.tile_pool(name="accA", bufs=3) as accA:
                x16 = xpool.tile([P, XPR, XPC], F16)
                wk0 = xpool.tile([P, NT5], F32)
                diag0 = xpool.tile([P, NT5, P], F16)

                nc.sync.dma_start(out=x16[:, :, :], in_=xh_d[:, :, :])

                # pooled1: copy pass with accumulate (junk dest = attn1 center,
                # overwritten later by the gelu writes)
                nc.vector.tensor_scalar(attn1[:, 9:9 + H, 9:9 + WH],
                                        x16[:, 2:2 + H, 2:2 + WH],
                                        1.0, 0.0, ALU.mult, ALU.add,
                                        accum_out=pool1raw[:, :])

                routing_chain(pool1raw, 1.0 / (H * W), r0wTsb, r0bsb, rsb0,
                              r0scr, r0bc, poolm)
                mix_weights(r0bc, wexp0sb, wk0)
                build_diags(diag0, wk0, NT5)

                # conv1 + gelu over h tiles
                for i in range(NTILES):
                    h0 = i * HTILE
                    if i in DVE_A:
                        acc = accA.tile([P, HTILE, WH], F32)
                        for t, (di, dj) in enumerate(TAPS5):
                            v = x16[:, h0 + di:h0 + di + HTILE, dj:dj + WH]
                            if t == 0:
                                nc.vector.tensor_scalar(acc[:, :, :], v,
                                                        wk0[:, 0:1], None, ALU.mult)
                            else:
                                nc.vector.scalar_tensor_tensor(
                                    acc[:, :, :], v, wk0[:, t:t + 1],
                                    acc[:, :, :], ALU.mult, ALU.add)
                        src = acc[:, :, :]
                    else:
                        ps = psumA.tile([P, HTILE, WH], F32)
                        for t, (di, dj) in enumerate(TAPS5):
                            v = x16[:, h0 + di:h0 + di + HTILE, dj:dj + WH]
                            nc.tensor.matmul(ps[:, :, :], lhsT=diag0[:, t, :],
                                             rhs=v, start=(t == 0),
                                             stop=(t == NT5 - 1))
                        src = ps[:, :, :]
                    nc.scalar.activation(
                        out=attn1[:, 9 + h0:9 + h0 + HTILE, 9:9 + WH], in_=src,
                        func=ACTF.Gelu, bias=b0sb[:, :], scale=1.0,
                        accum_out=stats1[:, i:i + 1])

            # attn1 cross-half halo exchange: gather strips to contiguous staging,
            # one fat cross-partition DMA, scatter into the halo columns.
            # wh=0 right halo <- wh=1 cols [9:18);  wh=1 left halo <- wh=0 cols [128:137)
            nc.vector.tensor_copy(hgat[C:P, :, :], attn1[C:P, 9:9 + H, 9:18])
            nc.vector.tensor_copy(hgat[0:C, :, :], attn1[0:C, 9:9 + H, 9 + WH - 9:9 + WH])
            nc.sync.dma_start(out=hswp[0:C, :, :], in_=hgat[C:P, :, :])
            nc.sync.dma_start(out=hswp[C:P, :, :], in_=hgat[0:C, :, :])
            nc.vector.tensor_copy(attn1[0:C, 9:9 + H, 9 + WH:18 + WH], hswp[0:C, :, :])
            nc.vector.tensor_copy(attn1[C:P, 9:9 + H, 0:9], hswp[C:P, :, :])

            # =================== routing 1, conv2, 1x1, gate ====================
            with tc.tile_pool(name="accB", bufs=3) as accB, \
                    tc.tile_pool(name="a2pool", bufs=3) as a2pool, \
                    tc.tile_pool(name="x32pool", bufs=4) as x32pool, \
                    tc.tile_pool(name="tpool", bufs=3) as tpool, \
                    tc.tile_pool(name="outpool", bufs=3) as outpool:

                nc.vector.tensor_reduce(pool2raw[:, :], stats1[:, :],
                                        axis=mybir.AxisListType.X, op=ALU.add)
                routing_chain(pool2raw, 1.0 / (H * W), r1wTsb, r1bsb, rsb1,
                              r1scr, r1bc, poolm2)
                mix_weights(r1bc, wexp1sb, wk1)
                build_diags(diag1, wk1, NT7)

                for i in range(NTILES):
                    h0 = i * HTILE
                    if i in DVE_B:
                        acc = accB.tile([P, HTILE, WH], F32)
                        for t, (di, dj) in enumerate(TAPS7):
                            v = attn1[:, h0 + 3 * di:h0 + 3 * di + HTILE,
                                      3 * dj:3 * dj + WH]
                            if t == 0:
                                nc.vector.tensor_scalar(acc[:, :, :], v,
                                                        wk1[:, 0:1], None, ALU.mult)
                            else:
                                nc.vector.scalar_tensor_tensor(
                                    acc[:, :, :], v, wk1[:, t:t + 1],
                                    acc[:, :, :], ALU.mult, ALU.add)
                        src = acc[:, :, :]
                    else:
                        ps = psumA.tile([P, HTILE, WH], F32)
                        for t, (di, dj) in enumerate(TAPS7):
                            v = attn1[:, h0 + 3 * di:h0 + 3 * di + HTILE,
                                      3 * dj:3 * dj + WH]
                            nc.tensor.matmul(ps[:, :, :], lhsT=diag1[:, t, :],
                                             rhs=v, start=(t == 0),
                                             stop=(t == NT7 - 1))
                        src = ps[:, :, :]

                    a2 = a2pool.tile([P, HTILE, WH], F16)
                    nc.scalar.activation(out=a2[:, :, :], in_=src, func=ACTF.Gelu,
                                         bias=b1sb[:, :], scale=1.0)

                    ps2 = psumB.tile([P, HTILE, WH], F32)
                    nc.tensor.matmul(ps2[:, :, :], lhsT=wpbdsb[:, :],
                                     rhs=a2[:, :, :], start=True, stop=True)

                    tsb = tpool.tile([P, HTILE, WH], F32)
                    nc.scalar.activation(out=tsb[:, :, :], in_=ps2[:, :, :],
                                         func=ACTF.Identity, bias=bpsb[:, :],
                                         scale=1.0)

                    x32 = x32pool.tile([P, HTILE, WH], F32)
                    nc.sync.dma_start(out=x32[:, :, :],
                                      in_=x32_d[:, h0:h0 + HTILE, :])

                    osb = outpool.tile([P, HTILE, WH], F32)
                    nc.vector.tensor_mul(osb[:, :, :], tsb[:, :, :], x32[:, :, :])

                    nc.sync.dma_start(out=out_d[:, h0:h0 + HTILE, :],
                                      in_=osb[:, :, :])

    nc.finalize()
    return nc


def _host_inputs(x, w0, b0, r0_w, r0_b, w1, b1, r1_w, r1_b, wp, bp):
    """Build the per-core input maps (core b gets sample b; weights shared)."""
    base0 = np.ascontiguousarray(w0[:, :, 0, :, :].reshape(K, C, NT5))
    wexp0 = np.ascontiguousarray(
        np.tile(base0.transpose(1, 0, 2), (2, 1, 1)), dtype=np.float32)
    base1 = np.ascontiguousarray(w1[:, :, 0, :, :].reshape(K, C, NT7))
    wexp1 = np.ascontiguousarray(
        np.tile(base1.transpose(1, 0, 2), (2, 1, 1)), dtype=np.float32)
    shared = {
        "wexp0": wexp0,
        "wexp1": wexp1,
        "r0wT": np.ascontiguousarray(r0_w.T, dtype=np.float32),
        "r1wT": np.ascontiguousarray(r1_w.T, dtype=np.float32),
        "r0b": np.ascontiguousarray(r0_b[:, None], dtype=np.float32),
        "r1b": np.ascontiguousarray(r1_b[:, None], dtype=np.float32),
        "s2": np.ascontiguousarray(np.tile(np.eye(C, dtype=np.float32), (2, 1))),
        "i128": np.eye(P, dtype=np.float16),
        "wpbd": np.kron(np.eye(2), wp.T).astype(np.float16),
        "b0r": np.ascontiguousarray(np.tile(b0, 2)[:, None], dtype=np.float32),
        "b1r": np.ascontiguousarray(np.tile(b1, 2)[:, None], dtype=np.float32),
        "bpr": np.ascontiguousarray(np.tile(bp, 2)[:, None], dtype=np.float32),
    }
    in_maps = []
    for b in range(NCORES):
        xs = x[b]                                   # [C, H, W] f32
        xp = np.pad(xs, ((0, 0), (2, 2), (2, 2))).astype(np.float16)
        xh = np.concatenate([xp[:, :, 0:XPC], xp[:, :, W - WH:W + 4]], axis=0)
        x32 = np.concatenate([xs[:, :, :WH], xs[:, :, WH:]], axis=0)
        m = dict(shared)
        m["xh"] = np.ascontiguousarray(xh)          # [128, 260, 132] f16
        m["x32"] = np.ascontiguousarray(x32, dtype=np.float32)
        in_maps.append(m)
    return in_maps


_CACHE_LOCK = threading.Lock()
_PROGRAM = None
LAST_RESULTS = None  # BassKernelResults of the most recent run (for test.py)


def _get_program():
    global _PROGRAM
    with _CACHE_LOCK:
        if _PROGRAM is None:
            _PROGRAM = _build_program()
    return _PROGRAM


def _timed_sharded_run(nc, in_maps, iters=6):
    """Time device-resident executions of the compiled program (mirrors
    bass2jax.run_bass_via_pjrt's sharded path, but stages inputs on device
    first so the timed window is just the NEFF execution + dispatch)."""
    import time

    import jax
    from jax.experimental.shard_map import shard_map
    from jax.sharding import Mesh, NamedSharding, PartitionSpec

    from concourse import bass2jax, mybir as _mybir

    bass2jax.install_neuronx_cc_hook()
    n_cores = len(in_maps)
    partition_name = nc.partition_id_tensor.name if nc.partition_id_tensor else None

    in_names, out_names, out_avals, zero_outs = [], [], [], []
    for alloc in nc.m.functions[0].allocations:
        if not isinstance(alloc, _mybir.MemoryLocationSet):
            continue
        name = alloc.memorylocations[0].name
        if alloc.kind == "ExternalInput":
            if name != partition_name:
                in_names.append(name)
        elif alloc.kind == "ExternalOutput":
            shape = tuple(alloc.tensor_shape)
            dtype = _mybir.dt.np(alloc.dtype)
            out_names.append(name)
            out_avals.append(jax.core.ShapedArray(shape, dtype))
            zero_outs.append(np.zeros(shape, dtype))
    n_params = len(in_names)
    n_outs = len(out_avals)
    all_in_names = list(in_names) + list(out_names)
    if partition_name is not None:
        all_in_names.append(partition_name)
    donate = tuple(range(n_params, n_params + n_outs))

    def _body(*args):
        operands = list(args)
        if partition_name is not None:
            operands.append(bass2jax.partition_id_tensor())
        return tuple(bass2jax._bass_exec_p.bind(
            *operands,
            out_avals=tuple(out_avals),
            in_names=tuple(all_in_names),
            out_names=tuple(out_names),
            lowering_input_output_aliases=(),
            sim_require_finite=True,
            sim_require_nnan=True,
            nc=nc,
        ))

    devices = jax.devices()[:n_cores]
    mesh = Mesh(np.asarray(devices), ("core",))
    sh = NamedSharding(mesh, PartitionSpec("core"))
    in_specs = (PartitionSpec("core"),) * (n_params + n_outs)
    out_specs = (PartitionSpec("core"),) * n_outs
    sharded = jax.jit(
        shard_map(_body, mesh=mesh, in_specs=in_specs, out_specs=out_specs,
                  check_rep=False),
        donate_argnums=donate, keep_unused=True)

    concat_in = [
        jax.device_put(
            np.concatenate([np.asarray(in_maps[c][nm]) for c in range(n_cores)],
                           axis=0), sh)
        for nm in in_names
    ]
    zero_concat = [np.concatenate([z] * n_cores, axis=0) for z in zero_outs]
    jax.block_until_ready(concat_in)

    times = []
    outs = None
    for _ in range(iters):
        zs = [jax.device_put(z, sh) for z in zero_concat]
        jax.block_until_ready(zs)
        t0 = time.perf_counter()
        outs = sharded(*concat_in, *zs)
        jax.block_until_ready(outs)
        times.append(time.perf_counter() - t0)
    return times, outs, out_names, (sharded, concat_in, zero_concat, sh)


def _timed_async_batch(ctx, batch=16, iters=3):
    """Queue `batch` executions back-to-back (async dispatch), block once.
    If dispatch pipelines, total ~= overhead + batch * body_time."""
    import time

    import jax

    sharded, concat_in, zero_concat, sh = ctx
    res = []
    for _ in range(iters):
        zss = [[jax.device_put(z, sh) for z in zero_concat]
               for _ in range(batch)]
        for zs in zss:
            jax.block_until_ready(zs)
        t0 = time.perf_counter()
        outs = [sharded(*concat_in, *zs) for zs in zss]
        jax.block_until_ready(outs)
        res.append(time.perf_counter() - t0)
    return res


def kernel(x, w0, b0, r0_w, r0_b, w1, b1, r1_w, r1_b, wp, bp,
           trace=False, **trace_kwargs):
    global LAST_RESULTS
    x = np.asarray(x, dtype=np.float32)
    nc = _get_program()
    in_maps = _host_inputs(x, np.asarray(w0), np.asarray(b0), np.asarray(r0_w),
                           np.asarray(r0_b), np.asarray(w1), np.asarray(b1),
                           np.asarray(r1_w), np.asarray(r1_b), np.asarray(wp),
                           np.asarray(bp))
    res = run_bass_kernel_spmd(nc, in_maps, core_ids=list(range(NCORES)),
                               trace=trace, **trace_kwargs)
    LAST_RESULTS = res
    out_full = np.empty((NCORES, C, H, W), dtype=np.float32)
    for b, r in enumerate(res.results):
        oc = r["out"]                               # [128, 256, 128]
        out_full[b, :, :, :WH] = oc[:C]
        out_full[b, :, :, WH:] = oc[C:]
    return out_full



# revision 2
# speedup vs baseline: 6.6993x; 6.6993x over previous
"""Trainium2 Bass kernel for dynamic-LKA (CondConv depthwise mix) module.

Reference computation (per sample):
  r0 = sigmoid(mean_hw(x) @ r0_w.T + r0_b)            # [K] routing
  wk0 = sum_k r0_k * w0[k]                            # mixed 5x5 depthwise kernel
  a1 = gelu(dwconv5x5(x, wk0, pad=2, dil=1) + b0)
  r1 = sigmoid(mean_hw(a1) @ r1_w.T + r1_b)
  wk1 = sum_k r1_k * w1[k]                            # mixed 7x7 dil3 kernel
  a2 = gelu(dwconv7x7d3(a1, wk1, pad=9, dil=3) + b1)
  attn = a2 conv1x1 wp + bp
  out = x * attn

Sharding: pure data parallel, 1 sample per NeuronCore (B=8 over 8 cores).

End-to-end wall time is dominated by host<->device transfer over the axon
tunnel, so I/O bytes are minimized: x ships once as f16 [C,H,W] (the padded
conv slab, halos, and the gate operand are all built on device from it) and
the output returns as f16, halving both the donated zero-output upload and
the result fetch.

Per-core device strategy:
  - Layout: partitions p = wh*64 + c (w-half, channel); free dims (h, w_local).
  - Depthwise conv taps run as PE matmuls with *diagonal* stationary
    matrices diag(wk[:, tap]) accumulating in PSUM; a fraction of h-tiles
    instead run on the DVE as fp32 scalar_tensor_tensor MAC chains so both
    engines stay busy.
  - gelu (+channel bias) runs on the ACT engine straight out of PSUM and
    its accum_out provides the per-partition sums for the second routing.
  - 1x1 conv is one PE matmul per tile with a block-diagonal wp.
  - Final gate multiply reads x from the resident f16 slab.
"""

import os
import sys
import threading

import numpy as np

for _p in ("/opt/trn_rl_repo",):
    if _p not in sys.path and os.path.isdir(_p):
        sys.path.insert(0, _p)

import concourse.bacc as bacc
import concourse.bass as bass
import concourse.mybir as mybir
import concourse.tile as tile
from concourse.bass_utils import run_bass_kernel_spmd

B, C, H, W = 8, 64, 256, 256
K = 3
NCORES = 8
WH = W // 2  # 128, per-partition w width
P = 128

F32 = mybir.dt.float32
F16 = mybir.dt.float16

TAPS5 = [(di, dj) for di in range(5) for dj in range(5)]   # conv1, offsets di-2, dj-2
TAPS7 = [(di, dj) for di in range(7) for dj in range(7)]   # conv2, offsets 3*(di-3), 3*(dj-3)
NT5, NT7 = len(TAPS5), len(TAPS7)

HTILE = 4                      # output h rows per tile -> N=512 moving columns
NTILES = H // HTILE            # 64

# x16 padded slab: 2 pad rows/cols each side (conv1 radius 2)
XPR, XPC = H + 4, WH + 4       # 260 x 132
# attn1 padded slab: 9 pad rows/cols each side (conv2 reach 9)
APR, APC = H + 18, WH + 18     # 274 x 146

# which tiles run on DVE instead of PE (load balancing)
DVE_A = frozenset(i for i in range(NTILES) if i % 15 in (1, 5, 9, 13))   # ~17
DVE_B = frozenset(i for i in range(NTILES) if i % 17 in (1, 5, 9, 13))   # ~15

ALU = mybir.AluOpType
ACTF = mybir.ActivationFunctionType


def _build_program():
    nc = bacc.Bacc(None, target_bir_lowering=False)

    # ---- kernel I/O ------------------------------------------------------
    x16_d = nc.dram_tensor("x16", [C, H, W], F16, kind="ExternalInput")
    wexp0_d = nc.dram_tensor("wexp0", [P, K, NT5], F32, kind="ExternalInput")
    wexp1_d = nc.dram_tensor("wexp1", [P, K, NT7], F32, kind="ExternalInput")
    r0wT_d = nc.dram_tensor("r0wT", [C, K], F32, kind="ExternalInput")
    r1wT_d = nc.dram_tensor("r1wT", [C, K], F32, kind="ExternalInput")
    r0b_d = nc.dram_tensor("r0b", [K, 1], F32, kind="ExternalInput")
    r1b_d = nc.dram_tensor("r1b", [K, 1], F32, kind="ExternalInput")
    s2_d = nc.dram_tensor("s2", [P, C], F32, kind="ExternalInput")
    i128_d = nc.dram_tensor("i128", [P, P], F16, kind="ExternalInput")
    wpbd_d = nc.dram_tensor("wpbd", [P, P], F16, kind="ExternalInput")
    b0_d = nc.dram_tensor("b0r", [P, 1], F32, kind="ExternalInput")
    b1_d = nc.dram_tensor("b1r", [P, 1], F32, kind="ExternalInput")
    bp_d = nc.dram_tensor("bpr", [P, 1], F32, kind="ExternalInput")
    out_d = nc.dram_tensor("out", [C, H, W], F16, kind="ExternalOutput")

    # DRAM bounce buffers for broadcasting routing weights to all partitions
    r0scr = nc.dram_tensor("r0scr", [K, 1], F32)
    r1scr = nc.dram_tensor("r1scr", [K, 1], F32)

    with tile.TileContext(nc) as tc, \
            tc.tile_pool(name="consts", bufs=1) as consts, \
            tc.tile_pool(name="a1pool", bufs=1) as a1pool, \
            tc.tile_pool(name="smalls", bufs=1) as smalls, \
            tc.tile_pool(name="psumA", bufs=4, space="PSUM") as psumA, \
            tc.tile_pool(name="psumB", bufs=2, space="PSUM") as psumB, \
            tc.tile_pool(name="psumT", bufs=1, space="PSUM") as psumT:

        # ---- constants ----------------------------------------------------
        s2sb = consts.tile([P, C], F32)
        nc.sync.dma_start(out=s2sb, in_=s2_d[:, :])
        i128sb = consts.tile([P, P], F16)
        nc.sync.dma_start(out=i128sb, in_=i128_d[:, :])
        wpbdsb = consts.tile([P, P], F16)
        nc.sync.dma_start(out=wpbdsb, in_=wpbd_d[:, :])
        b0sb = consts.tile([P, 1], F32)
        nc.sync.dma_start(out=b0sb, in_=b0_d[:, :])
        b1sb = consts.tile([P, 1], F32)
        nc.sync.dma_start(out=b1sb, in_=b1_d[:, :])
        bpsb = consts.tile([P, 1], F32)
        nc.sync.dma_start(out=bpsb, in_=bp_d[:, :])
        r0wTsb = consts.tile([C, K], F32)
        nc.sync.dma_start(out=r0wTsb, in_=r0wT_d[:, :])
        r1wTsb = consts.tile([C, K], F32)
        nc.sync.dma_start(out=r1wTsb, in_=r1wT_d[:, :])
        r0bsb = consts.tile([K, 1], F32)
        nc.sync.dma_start(out=r0bsb, in_=r0b_d[:, :])
        r1bsb = consts.tile([K, 1], F32)
        nc.sync.dma_start(out=r1bsb, in_=r1b_d[:, :])
        wexp0sb = consts.tile([P, K, NT5], F32)
        nc.sync.dma_start(out=wexp0sb, in_=wexp0_d[:, :, :])
        wexp1sb = consts.tile([P, K, NT7], F32)
        nc.sync.dma_start(out=wexp1sb, in_=wexp1_d[:, :, :])

        # x16 resident padded slab (fp16), 2-wide zero pads/halos
        xslab = a1pool.tile([P, XPR, XPC], F16)
        # attn1 resident slab (fp16), with 9-wide zero pads/halos
        attn1 = a1pool.tile([P, APR, APC], F16)
        nc.vector.memset(attn1[:, 0:9, :], 0.0)
        nc.vector.memset(attn1[:, APR - 9:APR, :], 0.0)
        nc.vector.memset(attn1[0:C, 9:APR - 9, 0:9], 0.0)          # wh=0 left edge
        nc.vector.memset(attn1[C:P, 9:APR - 9, APC - 9:APC], 0.0)  # wh=1 right edge

        stats1 = smalls.tile([P, NTILES], F32)
        pool1raw = smalls.tile([P, 1], F32)
        pool2raw = smalls.tile([P, 1], F32)
        poolm = smalls.tile([C, 1], F32)
        poolm2 = smalls.tile([C, 1], F32)
        rsb0 = smalls.tile([K, 1], F32)
        rsb1 = smalls.tile([K, 1], F32)
        r0bc = smalls.tile([P, K], F32)
        r1bc = smalls.tile([P, K], F32)
        wk1 = smalls.tile([P, NT7], F32)
        diag1 = smalls.tile([P, NT7, P], F16)
        hgat = smalls.tile([P, H, 9], F16)   # halo exchange staging (gather)
        hswp = smalls.tile([P, H, 9], F16)   # halo exchange staging (swapped)

        def routing_chain(poolraw, scale, rwTsb, rbsb, rsb, rscr_d, rbc, pm):
            """poolraw [P,1] -> r [K] -> broadcast to all partitions [P,K]."""
            ps1 = psumT.tile([C, 1], F32)
            nc.tensor.matmul(ps1[:, :], lhsT=s2sb[:, :], rhs=poolraw[:, :],
                             start=True, stop=True)
            nc.scalar.activation(out=pm[:, :], in_=ps1[:, :],
                                 func=ACTF.Copy, bias=0.0, scale=scale)
            ps2 = psumT.tile([K, 1], F32)
            nc.tensor.matmul(ps2[:, :], lhsT=rwTsb[:, :], rhs=pm[:, :],
                             start=True, stop=True)
            nc.scalar.activation(out=rsb[:, :], in_=ps2[:, :],
                                 func=ACTF.Sigmoid, bias=rbsb[:, :], scale=1.0)
            nc.sync.dma_start(out=rscr_d[:, :], in_=rsb[:, :])
            bcast = bass.AP(tensor=rscr_d, offset=0, ap=[[0, P], [1, K]])
            nc.gpsimd.dma_start(out=rbc[:, :], in_=bcast)

        def mix_weights(rbc, wexpsb, wk):
            nc.vector.tensor_scalar(wk[:, :], wexpsb[:, 0, :], rbc[:, 0:1], None,
                                    ALU.mult)
            for k in range(1, K):
                nc.vector.scalar_tensor_tensor(wk[:, :], wexpsb[:, k, :],
                                               rbc[:, k:k + 1], wk[:, :],
                                               ALU.mult, ALU.add)

        def build_diags(diag, wk, ntaps):
            for t in range(ntaps):
                nc.vector.tensor_scalar(diag[:, t, :], i128sb[:, :],
                                        wk[:, t:t + 1], None, ALU.mult)

        # ============ phase 1: load x, routing 0, conv1 ====================
        with tc.tile_pool(name="xpool", bufs=1) as xpool, \
                tc.tile_pool(name="accA", bufs=3) as accA:
            wk0 = xpool.tile([P, NT5], F32)
            diag0 = xpool.tile([P, NT5, P], F16)

            # build the padded slab from the unpadded [C,H,W] input:
            # zero borders, two half-width DMAs, then a 2-wide cross-half
            # halo exchange (gather -> cross-partition DMA -> scatter).
            nc.vector.memset(xslab[:, 0:2, :], 0.0)
            nc.vector.memset(xslab[:, XPR - 2:XPR, :], 0.0)
            nc.vector.memset(xslab[0:C, 2:XPR - 2, 0:2], 0.0)
            nc.vector.memset(xslab[C:P, 2:XPR - 2, XPC - 2:XPC], 0.0)
            nc.sync.dma_start(out=xslab[0:C, 2:2 + H, 2:2 + WH],
                              in_=x16_d[:, :, 0:WH])
            nc.sync.dma_start(out=xslab[C:P, 2:2 + H, 2:2 + WH],
                              in_=x16_d[:, :, WH:W])
            nc.vector.tensor_copy(hgat[C:P, :, 0:2], xslab[C:P, 2:2 + H, 2:4])
            nc.vector.tensor_copy(hgat[0:C, :, 0:2],
                                  xslab[0:C, 2:2 + H, WH:2 + WH])
            nc.sync.dma_start(out=hswp[0:C, :, 0:2], in_=hgat[C:P, :, 0:2])
            nc.sync.dma_start(out=hswp[C:P, :, 0:2], in_=hgat[0:C, :, 0:2])
            nc.vector.tensor_copy(xslab[0:C, 2:2 + H, 2 + WH:4 + WH],
                                  hswp[0:C, :, 0:2])
            nc.vector.tensor_copy(xslab[C:P, 2:2 + H, 0:2], hswp[C:P, :, 0:2])

            # pooled1: copy pass with accumulate (junk dest = attn1 center,
            # overwritten later by the gelu writes)
            nc.vector.tensor_scalar(attn1[:, 9:9 + H, 9:9 + WH],
                                    xslab[:, 2:2 + H, 2:2 + WH],
                                    1.0, 0.0, ALU.mult, ALU.add,
                                    accum_out=pool1raw[:, :])

            routing_chain(pool1raw, 1.0 / (H * W), r0wTsb, r0bsb, rsb0,
                          r0scr, r0bc, poolm)
            mix_weights(r0bc, wexp0sb, wk0)
            build_diags(diag0, wk0, NT5)

            # conv1 + gelu over h tiles
            for i in range(NTILES):
                h0 = i * HTILE
                if i in DVE_A:
                    acc = accA.tile([P, HTILE, WH], F32)
                    for t, (di, dj) in enumerate(TAPS5):
                        v = xslab[:, h0 + di:h0 + di + HTILE, dj:dj + WH]
                        if t == 0:
                            nc.vector.tensor_scalar(acc[:, :, :], v,
                                                    wk0[:, 0:1], None, ALU.mult)
                        else:
                            nc.vector.scalar_tensor_tensor(
                                acc[:, :, :], v, wk0[:, t:t + 1],
                                acc[:, :, :], ALU.mult, ALU.add)
                    src = acc[:, :, :]
                else:
                    ps = psumA.tile([P, HTILE, WH], F32)
                    for t, (di, dj) in enumerate(TAPS5):
                        v = xslab[:, h0 + di:h0 + di + HTILE, dj:dj + WH]
                        nc.tensor.matmul(ps[:, :, :], lhsT=diag0[:, t, :],
                                         rhs=v, start=(t == 0),
                                         stop=(t == NT5 - 1))
                    src = ps[:, :, :]
                nc.scalar.activation(
                    out=attn1[:, 9 + h0:9 + h0 + HTILE, 9:9 + WH], in_=src,
                    func=ACTF.Gelu, bias=b0sb[:, :], scale=1.0,
                    accum_out=stats1[:, i:i + 1])

        # attn1 cross-half halo exchange: gather strips to contiguous staging,
        # one fat cross-partition DMA, scatter into the halo columns.
        # wh=0 right halo <- wh=1 cols [9:18);  wh=1 left halo <- wh=0 cols [128:137)
        nc.vector.tensor_copy(hgat[C:P, :, :], attn1[C:P, 9:9 + H, 9:18])
        nc.vector.tensor_copy(hgat[0:C, :, :], attn1[0:C, 9:9 + H, 9 + WH - 9:9 + WH])
        nc.sync.dma_start(out=hswp[0:C, :, :], in_=hgat[C:P, :, :])
        nc.sync.dma_start(out=hswp[C:P, :, :], in_=hgat[0:C, :, :])
        nc.vector.tensor_copy(attn1[0:C, 9:9 + H, 9 + WH:18 + WH], hswp[0:C, :, :])
        nc.vector.tensor_copy(attn1[C:P, 9:9 + H, 0:9], hswp[C:P, :, :])

        # =================== routing 1, conv2, 1x1, gate ====================
        with tc.tile_pool(name="accB", bufs=3) as accB, \
                tc.tile_pool(name="a2pool", bufs=3) as a2pool, \
                tc.tile_pool(name="tpool", bufs=3) as tpool, \
                tc.tile_pool(name="outpool", bufs=3) as outpool:

            nc.vector.tensor_reduce(pool2raw[:, :], stats1[:, :],
                                    axis=mybir.AxisListType.X, op=ALU.add)
            routing_chain(pool2raw, 1.0 / (H * W), r1wTsb, r1bsb, rsb1,
                          r1scr, r1bc, poolm2)
            mix_weights(r1bc, wexp1sb, wk1)
            build_diags(diag1, wk1, NT7)

            for i in range(NTILES):
                h0 = i * HTILE
                if i in DVE_B:
                    acc = accB.tile([P, HTILE, WH], F32)
                    for t, (di, dj) in enumerate(TAPS7):
                        v = attn1[:, h0 + 3 * di:h0 + 3 * di + HTILE,
                                  3 * dj:3 * dj + WH]
                        if t == 0:
                            nc.vector.tensor_scalar(acc[:, :, :], v,
                                                    wk1[:, 0:1], None, ALU.mult)
                        else:
                            nc.vector.scalar_tensor_tensor(
                                acc[:, :, :], v, wk1[:, t:t + 1],
                                acc[:, :, :], ALU.mult, ALU.add)
                    src = acc[:, :, :]
                else:
                    ps = psumA.tile([P, HTILE, WH], F32)
                    for t, (di, dj) in enumerate(TAPS7):
                        v = attn1[:, h0 + 3 * di:h0 + 3 * di + HTILE,
                                  3 * dj:3 * dj + WH]
                        nc.tensor.matmul(ps[:, :, :], lhsT=diag1[:, t, :],
                                         rhs=v, start=(t == 0),
                                         stop=(t == NT7 - 1))
                    src = ps[:, :, :]

                a2 = a2pool.tile([P, HTILE, WH], F16)
                nc.scalar.activation(out=a2[:, :, :], in_=src, func=ACTF.Gelu,
                                     bias=b1sb[:, :], scale=1.0)

                ps2 = psumB.tile([P, HTILE, WH], F32)
                nc.tensor.matmul(ps2[:, :, :], lhsT=wpbdsb[:, :],
                                 rhs=a2[:, :, :], start=True, stop=True)

                tsb = tpool.tile([P, HTILE, WH], F32)
                nc.scalar.activation(out=tsb[:, :, :], in_=ps2[:, :, :],
                                     func=ACTF.Identity, bias=bpsb[:, :],
                                     scale=1.0)

                osb = outpool.tile([P, HTILE, WH], F16)
                nc.vector.tensor_mul(osb[:, :, :], tsb[:, :, :],
                                     xslab[:, 2 + h0:2 + h0 + HTILE, 2:2 + WH])

                nc.sync.dma_start(out=out_d[:, h0:h0 + HTILE, 0:WH],
                                  in_=osb[0:C, :, :])
                nc.sync.dma_start(out=out_d[:, h0:h0 + HTILE, WH:W],
                                  in_=osb[C:P, :, :])

    nc.finalize()
    return nc


def _host_inputs(x, w0, b0, r0_w, r0_b, w1, b1, r1_w, r1_b, wp, bp):
    """Build the per-core input maps (core b gets sample b; weights shared)."""
    base0 = np.ascontiguousarray(w0[:, :, 0, :, :].reshape(K, C, NT5))
    wexp0 = np.ascontiguousarray(
        np.tile(base0.transpose(1, 0, 2), (2, 1, 1)), dtype=np.float32)
    base1 = np.ascontiguousarray(w1[:, :, 0, :, :].reshape(K, C, NT7))
    wexp1 = np.ascontiguousarray(
        np.tile(base1.transpose(1, 0, 2), (2, 1, 1)), dtype=np.float32)
    shared = {
        "wexp0": wexp0,
        "wexp1": wexp1,
        "r0wT": np.ascontiguousarray(r0_w.T, dtype=np.float32),
        "r1wT": np.ascontiguousarray(r1_w.T, dtype=np.float32),
        "r0b": np.ascontiguousarray(r0_b[:, None], dtype=np.float32),
        "r1b": np.ascontiguousarray(r1_b[:, None], dtype=np.float32),
        "s2": np.ascontiguousarray(np.tile(np.eye(C, dtype=np.float32), (2, 1))),
        "i128": np.eye(P, dtype=np.float16),
        "wpbd": np.kron(np.eye(2), wp.T).astype(np.float16),
        "b0r": np.ascontiguousarray(np.tile(b0, 2)[:, None], dtype=np.float32),
        "b1r": np.ascontiguousarray(np.tile(b1, 2)[:, None], dtype=np.float32),
        "bpr": np.ascontiguousarray(np.tile(bp, 2)[:, None], dtype=np.float32),
    }
    x16 = np.ascontiguousarray(x, dtype=np.float16)     # one vectorized cast
    in_maps = []
    for b in range(NCORES):
        m = dict(shared)
        m["x16"] = x16[b]                               # contiguous view
        in_maps.append(m)
    return in_maps


_CACHE_LOCK = threading.Lock()
_PROGRAM = None
LAST_RESULTS = None  # BassKernelResults of the most recent run (for test.py)


def _get_program():
    global _PROGRAM
    with _CACHE_LOCK:
        if _PROGRAM is None:
            _PROGRAM = _build_program()
    return _PROGRAM


def kernel(x, w0, b0, r0_w, r0_b, w1, b1, r1_w, r1_b, wp, bp,
           trace=False, **trace_kwargs):
    global LAST_RESULTS
    x = np.asarray(x, dtype=np.float32)
    nc = _get_program()
    in_maps = _host_inputs(x, np.asarray(w0), np.asarray(b0), np.asarray(r0_w),
                           np.asarray(r0_b), np.asarray(w1), np.asarray(b1),
                           np.asarray(r1_w), np.asarray(r1_b), np.asarray(wp),
                           np.asarray(bp))
    res = run_bass_kernel_spmd(nc, in_maps, core_ids=list(range(NCORES)),
                               trace=trace, **trace_kwargs)
    LAST_RESULTS = res
    out_full = np.empty((NCORES, C, H, W), dtype=np.float32)
    for b, r in enumerate(res.results):
        out_full[b] = r["out"]                          # f16 -> f32 cast
    return out_full
